# revision 13
# baseline (speedup 1.0000x reference)
"""EdgeGuidance Trainium2 kernel.

Pipeline per image [3,544,960] -> [1,136,240]:
  gray = w.RGB  ->  smooth = gauss5x5(reflect)  ->  gx,gy = sobel(zero-pad)
  mag = sqrt(gx^2+gy^2+1e-6)  ->  4x4 avgpool  ->  sigmoid(5(x-0.2))^2

All linear steps fold into two banded-matrix passes on the PE (f32r;
the mag -> 4x-pool tail runs in bf16):
  gx = A_x @ gray @ Bx^T,   gy = A_y @ gray @ By^T
Phase A uses gray as the matmul stationary so its output lands transposed
([w, s]); each of 5 row-blocks owns a disjoint s-window (rows overlap by 6
so no cross-block PSUM accumulation is needed). Phase B contracts over w
with the B^T band stationary.

DMA: every DRAM->SBUF load uses a multiple-of-8 partition count — the
HWDGE only sprays descriptors across all 16 SDMA engines in that case
(122-row loads pin to 1-2 engines and serialize at ~25 GB/s).

Data parallel over batch: 8 cores x 2 images.
"""

import numpy as np
import ml_dtypes

import concourse.bass as bass
import concourse.tile as tile
from concourse import mybir
from concourse.bass_utils import run_bass_kernel_spmd

F32 = mybir.dt.float32
F32R = mybir.dt.float32r
BF16 = mybir.dt.bfloat16
AF = mybir.ActivationFunctionType
ALU = mybir.AluOpType

B_FULL, C, H, W = 16, 3, 544, 960
N_CORES = 8
B_LOC = B_FULL // N_CORES  # images per core
HP, WP = H // 4, W // 4  # 136, 240

BLUR_K, SIGMA = 5, 1.5
W_R, W_G, W_B = 0.2989, 0.587, 0.114

# 5 gray row-blocks (k multiple of 8 for DMA engine spray), each owning a
# disjoint s-window; rows [s-3, s+4) of every owned s lie inside the block.
# Windows 0-3 equal (114) so copies can stride uniformly across slots.
GB = [(0, 120), (111, 231), (225, 345), (339, 459), (448, 544)]
SW = [(0, 114), (114, 228), (228, 342), (342, 456), (456, 544)]
N_WC = 8  # w-chunks of 120 outputs each


def _wj(j):
    return max(0, 120 * j - 4), min(W, 120 * j + 124)


# ---------------------------------------------------------------- numpy bands
def _blur1d():
    x = np.arange(BLUR_K, dtype=np.float64) - (BLUR_K - 1) / 2.0
    g = np.exp(-(x**2) / (2.0 * SIGMA**2))
    return g / g.sum()


def _band_reflect(n, taps):
    r = len(taps) // 2
    m = np.zeros((n, n), dtype=np.float64)
    for s in range(n):
        for d in range(-r, r + 1):
            i = s + d
            if i < 0:
                i = -i
            elif i >= n:
                i = 2 * n - 2 - i
            m[s, i] += taps[d + r]
    return m


def _band_zero(n, taps):
    r = len(taps) // 2
    m = np.zeros((n, n), dtype=np.float64)
    for s in range(n):
        for d in range(-r, r + 1):
            i = s + d
            if 0 <= i < n:
                m[s, i] += taps[d + r]
    return m


def build_constants():
    bf16 = ml_dtypes.bfloat16
    g1 = _blur1d()
    vb_h = _band_reflect(H, g1)  # vertical blur on H
    hb_w = _band_reflect(W, g1)  # horizontal blur on W
    ax = _band_zero(H, [1.0, 2.0, 1.0]) @ vb_h
    ay = _band_zero(H, [-1.0, 0.0, 1.0]) @ vb_h
    bx = _band_zero(W, [-1.0, 0.0, 1.0]) @ hb_w
    by = _band_zero(W, [1.0, 2.0, 1.0]) @ hb_w
    # fold gray scale W_R into the vertical bands (gray' = R + aG + bB)
    ax *= W_R
    ay *= W_R

    # phase A: slot i at col 256*i, interleaved (s, t): col 256i+2u+t
    band_a = np.zeros((128, 1280), dtype=np.float64)
    for i, ((r0, r1), (s0, s1)) in enumerate(zip(GB, SW)):
        k, w = r1 - r0, s1 - s0
        blk = np.stack([ax[s0:s1, r0:r1], ay[s0:s1, r0:r1]], axis=-1)  # [w,k,2]
        band_a[0:k, 256 * i : 256 * i + 2 * w] = blk.transpose(1, 0, 2).reshape(
            k, 2 * w
        )

    # phase B: per (t, j) block [mj, 120] at cols (t*8+j)*120
    band_b = np.zeros((128, 2 * N_WC * 120), dtype=np.float64)
    for t, m in enumerate((bx, by)):
        for j in range(N_WC):
            w0, w1 = _wj(j)
            blk = m[120 * j : 120 * j + 120, w0:w1].T  # [mj, 120]
            band_b[0 : w1 - w0, (t * N_WC + j) * 120 : (t * N_WC + j + 1) * 120] = blk

    p4 = np.zeros((128, 30), dtype=np.float64)
    for wp in range(120):
        p4[wp, wp // 4] = 1.0 / 16.0
    return (
        band_a.astype(np.float32),
        band_b.astype(np.float32),
        p4.astype(bf16),
    )


# ------------------------------------------------------------------ bass build
def split_multi_waits(nc):
    """walrus in this container only accepts 1 sync-wait per instruction;
    hoist extra waits onto preceding same-engine NoOps."""
    for fn in nc.m.functions:
        for bb in fn.blocks:
            new_list, changed = [], False
            for ins in bb.instructions:
                si = ins.sync_info
                waits = list(si.on_wait) if si is not None else []
                if len(waits) > 1:
                    changed = True
                    for i, wt in enumerate(waits[:-1]):
                        new_list.append(
                            mybir.InstNoOp(
                                name=f"{ins.name}_ws{i}",
                                engine=ins.engine,
                                bass_nofuse=True,
                                sync_info=mybir.SyncInfo(on_wait=[wt], on_update=[]),
                            )
                        )
                    si.on_wait = [waits[-1]]
                    ins.sync_info = si
                new_list.append(ins)
            if changed:
                bb.instructions = new_list


def build_module():
    nc = bass.Bass("TRN2", target_bir_lowering=False, debug=False)
    x = nc.dram_tensor("x", [B_LOC, C, H, W], F32, kind="ExternalInput").ap()
    ba = nc.dram_tensor("bA", [128, 1280], F32, kind="ExternalInput").ap()
    bb_ = nc.dram_tensor("bB", [128, 2 * N_WC * 120], F32, kind="ExternalInput").ap()
    p4 = nc.dram_tensor("p4", [128, 30], BF16, kind="ExternalInput").ap()
    y = nc.dram_tensor("y", [B_LOC, 1, HP, WP], F32, kind="ExternalOutput").ap()

    with tile.TileContext(nc) as tc:
        with (
            tc.tile_pool(name="const", bufs=1) as cpool,
            tc.tile_pool(name="rgb", bufs=6) as rgbp,
            tc.tile_pool(name="t1", bufs=2) as t1p,
            tc.tile_pool(name="gray", bufs=2) as grayp,
            tc.tile_pool(name="xy", bufs=4) as xyp,
            tc.tile_pool(name="sq", bufs=2) as sqp,
            tc.tile_pool(name="sp", bufs=2) as spp,
            tc.tile_pool(name="outp", bufs=2) as outp,
            tc.tile_pool(name="psA1", bufs=2, space="PSUM") as psA1,
            tc.tile_pool(name="psA2", bufs=1, space="PSUM") as psA2,
            tc.tile_pool(name="psB", bufs=1, space="PSUM") as psB,
            tc.tile_pool(name="psP", bufs=1, space="PSUM") as psP,
            nc.allow_low_precision(reason="bf16 pipeline, tolerance 2e-2"),
        ):
            # ---- constants: DMA in, round to f32r (verifier requires
            # f32r-matmul inputs to be produced as rounded f32r)
            ba_raw = cpool.tile([128, 1280], F32, tag="ba_raw")
            nc.sync.dma_start(ba_raw[:], ba[:])
            ba_t = cpool.tile([128, 1280], F32, tag="ba")
            nc.vector.tensor_copy(ba_t[:].bitcast(F32R), ba_raw[:])
            bb_raw = cpool.tile([128, 2 * N_WC * 120], F32, tag="bb_raw")
            nc.sync.dma_start(bb_raw[:], bb_[:])
            bb_t = cpool.tile([128, 2 * N_WC * 120], F32, tag="bb")
            nc.vector.tensor_copy(bb_t[:].bitcast(F32R), bb_raw[:])
            p4_t = cpool.tile([128, 30], BF16, tag="p4")
            nc.sync.dma_start(p4_t[:], p4[:])

            bias_eps = cpool.tile([128, 1], F32, tag="beps")
            nc.gpsimd.memset(bias_eps[:], 1e-6)
            bias_m1 = cpool.tile([128, 1], F32, tag="bm1")
            nc.gpsimd.memset(bias_m1[:], -1.0)

            stores = []
            # ---- W-halved units: 4 units/core = (image, half). Each half
            # loads [rows, 484] (480 + 4-col overlap) so the first unit's
            # load head is ~10us instead of ~20us; units pipeline flatly.
            WBASE = [0, 476]
            WC = 484

            def emit_loads(u):
                b, h = divmod(u, 2)
                rgbs = []
                for i, (r0, r1) in enumerate(GB):
                    k = r1 - r0
                    rgb = rgbp.tile([128, 3 * WC], F32, tag="rgb")
                    for c in range(3):
                        nc.sync.dma_start(
                            rgb[0:k, c * WC : (c + 1) * WC],
                            x[b, c, r0:r1, WBASE[h] : WBASE[h] + WC],
                        )
                    rgbs.append(rgb)
                return rgbs

            def emit_grays(u, rgbs):
                gs = []
                for i, (r0, r1) in enumerate(GB):
                    k = r1 - r0
                    rgb = rgbs[i]
                    tr = rgb[0:k, 0:WC]
                    tg = rgb[0:k, WC : 2 * WC]
                    tb = rgb[0:k, 2 * WC : 3 * WC]
                    t1 = t1p.tile([128, WC], F32, tag="t1")
                    nc.vector.scalar_tensor_tensor(
                        t1[0:k, :], tg, W_G / W_R, tr, op0=ALU.mult, op1=ALU.add
                    )
                    gt = grayp.tile([128, WC], F32, tag=f"g{i}")
                    nc.vector.scalar_tensor_tensor(
                        gt[0:k, :].bitcast(F32R), tb, W_B / W_R, t1[0:k, :],
                        op0=ALU.mult, op1=ALU.add,
                    )
                    gs.append(gt)
                return gs

            grays = {}
            pooled_t = {}

            def stage_a(c):
                u = c // 4
                b, h = divmod(u, 2)
                jj = c % 8
                w0, w1 = _wj(jj)
                w0l, w1l = w0 - WBASE[h], w1 - WBASE[h]
                mj = w1 - w0
                a01 = psA1.tile([128, 512], F32, tag="a01")
                a234 = psA2.tile([128, 768], F32, tag="a234")
                for i, (r0, r1) in enumerate(GB):
                    k = r1 - r0
                    dst = a01 if i < 2 else a234
                    off = 256 * i if i < 2 else 256 * (i - 2)
                    nc.tensor.matmul(
                        dst[0:mj, off : off + 256],
                        grays[u][i][0:k, w0l:w1l].bitcast(F32R),
                        ba_t[0:k, 256 * i : 256 * (i + 1)].bitcast(F32R),
                        start=True,
                        stop=True,
                    )
                return a01, a234

            def stage_copy(c, a01, a234):
                jj = c % 8
                w0, w1 = _wj(jj)
                mj = w1 - w0
                xy = xyp.tile([128, 1152], F32, tag="xy")
                nc.vector.tensor_copy(
                    xy[0:mj, 0:456].rearrange("p (b c) -> p b c", b=2)
                        .bitcast(F32R),
                    a01[0:mj, :].rearrange("p (b c) -> p b c", b=2)[:, :, 0:228],
                )
                nc.scalar.copy(
                    xy[0:mj, 456:1140].rearrange("p (b c) -> p b c", b=3)
                        .bitcast(F32R),
                    a234[0:mj, :].rearrange("p (b c) -> p b c", b=3)[:, :, 0:228],
                )
                return xy

            def stage_b(c, xy):
                jj = c % 8
                w0, w1 = _wj(jj)
                mj = w1 - w0
                xyv = xy[0:mj, 0:2 * H].rearrange("p (s two) -> p two s", two=2)
                gxy = psB.tile([128, 1312], F32, tag="gxy")
                for t, base in ((0, 224), (1, 768)):
                    bT = bb_t[
                        0:mj, (t * N_WC + jj) * 120 : (t * N_WC + jj + 1) * 120
                    ].bitcast(F32R)
                    n0 = 288 if t == 0 else 256
                    nc.tensor.matmul(
                        gxy[0:120, base : base + n0],
                        bT,
                        xyv[:, t, 0:n0].bitcast(F32R),
                        start=True,
                        stop=True,
                    )
                    nc.tensor.matmul(
                        gxy[0:120, base + n0 : base + 544],
                        bT,
                        xyv[:, t, n0:H].bitcast(F32R),
                        start=True,
                        stop=True,
                    )
                return gxy

            def stage_mag(c, gxy):
                sq = sqp.tile([128, 1088], F32, tag="sq")
                nc.scalar.activation(sq[0:120, :], gxy[0:120, 224:1312], AF.Square)
                m2 = sqp.tile([128, H], F32, tag="m2")
                nc.gpsimd.tensor_add(
                    m2[0:120, :], sq[0:120, 0:H], sq[0:120, H : 2 * H]
                )
                mg = sqp.tile([128, H], BF16, tag="mg")
                nc.scalar.activation(
                    mg[0:120, :], m2[0:120, :], AF.Sqrt, bias=bias_eps[0:120, :]
                )
                sp = spp.tile([128, HP], BF16, tag="sp")
                nc.vector.tensor_reduce(
                    sp[0:120, :],
                    mg[0:120, :].rearrange("p (g f) -> p g f", f=4),
                    axis=mybir.AxisListType.X,
                    op=ALU.add,
                )
                return sp

            def stage_pool(c, sp):
                b = c // 8
                jj = c % 8
                pooled = pooled_t[b]
                nc.tensor.matmul(
                    pooled[0:96, 30 * jj : 30 * jj + 30],
                    sp[0:120, 0:96],
                    p4_t[0:120, :],
                    start=True,
                    stop=True,
                )
                nc.tensor.matmul(
                    pooled[0:40, WP + 30 * jj : WP + 30 * jj + 30],
                    sp[0:120, 96:HP],
                    p4_t[0:120, :],
                    start=True,
                    stop=True,
                )

            def emit_sigmoid(b):
                sg = outp.tile([128, 2 * WP], F32, tag="sg")
                nc.scalar.activation(
                    sg[0:96, :], pooled_t[b][0:96, :], AF.Sigmoid,
                    bias=bias_m1[0:96, :], scale=5.0,
                )
                ot = outp.tile([128, 2 * WP], F32, tag="ot")
                nc.gpsimd.tensor_mul(ot[0:96, :], sg[0:96, :], sg[0:96, :])
                stores.append((b, ot))

            NU = 2 * B_LOC
            NC_ = 4 * NU
            rgbs_pend = {0: emit_loads(0)}
            grays[0] = emit_grays(0, rgbs_pend.pop(0))
            pooled_t[0] = psP.tile([128, 2 * WP], F32, tag="pooled", name="pooled0")
            aout = {0: stage_a(0)}
            sps, gxys = {}, {}
            for c in range(NC_):
                u = c // 4
                if c % 8 == 0 and c > 0:
                    pooled_t[c // 8] = psP.tile(
                        [128, 2 * WP], F32, tag="pooled", name=f"pooled{c // 8}"
                    )
                if c % 4 == 0 and u + 1 < NU:
                    rgbs_pend[u + 1] = emit_loads(u + 1)
                xy = stage_copy(c, *aout.pop(c))
                if c - 1 >= 0:
                    sps[c - 1] = stage_mag(c - 1, gxys.pop(c - 1))
                if c % 4 == 2 and u + 1 < NU:
                    grays[u + 1] = emit_grays(u + 1, rgbs_pend.pop(u + 1))
                if c + 1 < NC_:
                    aout[c + 1] = stage_a(c + 1)
                gxys[c] = stage_b(c, xy)
                if c - 2 >= 0:
                    stage_pool(c - 2, sps.pop(c - 2))
                    if (c - 2) % 8 == 7:
                        emit_sigmoid((c - 2) // 8)
            sps[NC_ - 1] = stage_mag(NC_ - 1, gxys.pop(NC_ - 1))
            for c in (NC_ - 2, NC_ - 1):
                stage_pool(c, sps.pop(c))
            emit_sigmoid(B_LOC - 1)

            # output stores last so image-1 input DMAs aren't blocked behind
            # image-0's store wait in the SP instruction stream
            for b, ot in stores:
                nc.sync.dma_start(y[b, 0, 0:96, :], ot[0:96, 0:WP])
                nc.sync.dma_start(y[b, 0, 96:HP, :], ot[0:40, WP : 2 * WP])

    split_multi_waits(nc)
    return nc


_NC = None
_CONSTS = None
TRACE = False
LAST_EXEC_NS = None


def kernel(**inputs):
    global _NC, _CONSTS, LAST_EXEC_NS
    left_rgb = np.ascontiguousarray(np.asarray(inputs["left_rgb"], dtype=np.float32))
    assert left_rgb.shape == (B_FULL, C, H, W)
    if _NC is None:
        _NC = build_module()
        _CONSTS = build_constants()
    band_a, band_b, p4 = _CONSTS
    in_maps = [
        {
            "x": np.ascontiguousarray(left_rgb[i * B_LOC : (i + 1) * B_LOC]),
            "bA": band_a,
            "bB": band_b,
            "p4": p4,
        }
        for i in range(N_CORES)
    ]
    res = run_bass_kernel_spmd(
        _NC, in_maps, core_ids=list(range(N_CORES)), trace=TRACE
    )
    LAST_EXEC_NS = res.exec_time_ns
    out = np.empty((B_FULL, 1, HP, WP), dtype=np.float32)
    for i in range(N_CORES):
        out[i * B_LOC : (i + 1) * B_LOC] = res.results[i]["y"]
    return out


# revision 15
# speedup vs baseline: 1.0889x; 1.0889x over previous
"""EdgeGuidance Trainium2 kernel.

Pipeline per image [3,544,960] -> [1,136,240]:
  gray = w.RGB  ->  smooth = gauss5x5(reflect)  ->  gx,gy = sobel(zero-pad)
  mag = sqrt(gx^2+gy^2+1e-6)  ->  4x4 avgpool  ->  sigmoid(5(x-0.2))^2

All linear steps fold into two banded-matrix passes on the PE (f32r;
the mag -> 4x-pool tail runs in bf16):
  gx = A_x @ gray @ Bx^T,   gy = A_y @ gray @ By^T
Phase A uses gray as the matmul stationary so its output lands transposed
([w, s]); each of 5 row-blocks owns a disjoint s-window (rows overlap by 6
so no cross-block PSUM accumulation is needed). Phase B contracts over w
with the B^T band stationary.

DMA: every DRAM->SBUF load uses a multiple-of-8 partition count — the
HWDGE only sprays descriptors across all 16 SDMA engines in that case
(122-row loads pin to 1-2 engines and serialize at ~25 GB/s).

Data parallel over batch: 8 cores x 2 images.
"""

import numpy as np
import ml_dtypes

import concourse.bass as bass
import concourse.tile as tile
from concourse import mybir
from concourse.bass_utils import run_bass_kernel_spmd

F32 = mybir.dt.float32
F32R = mybir.dt.float32r
BF16 = mybir.dt.bfloat16
AF = mybir.ActivationFunctionType
ALU = mybir.AluOpType

B_FULL, C, H, W = 16, 3, 544, 960
N_CORES = 8
B_LOC = B_FULL // N_CORES  # images per core
HP, WP = H // 4, W // 4  # 136, 240

BLUR_K, SIGMA = 5, 1.5
W_R, W_G, W_B = 0.2989, 0.587, 0.114

# 5 gray row-blocks (k multiple of 8 for DMA engine spray), each owning a
# disjoint s-window; rows [s-3, s+4) of every owned s lie inside the block.
# Windows 0-3 equal (114) so copies can stride uniformly across slots.
GB = [(0, 120), (111, 231), (225, 345), (339, 459), (448, 544)]
SW = [(0, 114), (114, 228), (228, 342), (342, 456), (456, 544)]
N_WC = 8  # w-chunks of 120 outputs each


def _wj(j):
    return max(0, 120 * j - 4), min(W, 120 * j + 124)


# ---------------------------------------------------------------- numpy bands
def _blur1d():
    x = np.arange(BLUR_K, dtype=np.float64) - (BLUR_K - 1) / 2.0
    g = np.exp(-(x**2) / (2.0 * SIGMA**2))
    return g / g.sum()


def _band_reflect(n, taps):
    r = len(taps) // 2
    m = np.zeros((n, n), dtype=np.float64)
    for s in range(n):
        for d in range(-r, r + 1):
            i = s + d
            if i < 0:
                i = -i
            elif i >= n:
                i = 2 * n - 2 - i
            m[s, i] += taps[d + r]
    return m


def _band_zero(n, taps):
    r = len(taps) // 2
    m = np.zeros((n, n), dtype=np.float64)
    for s in range(n):
        for d in range(-r, r + 1):
            i = s + d
            if 0 <= i < n:
                m[s, i] += taps[d + r]
    return m


def build_constants():
    bf16 = ml_dtypes.bfloat16
    g1 = _blur1d()
    vb_h = _band_reflect(H, g1)  # vertical blur on H
    hb_w = _band_reflect(W, g1)  # horizontal blur on W
    ax = _band_zero(H, [1.0, 2.0, 1.0]) @ vb_h
    ay = _band_zero(H, [-1.0, 0.0, 1.0]) @ vb_h
    bx = _band_zero(W, [-1.0, 0.0, 1.0]) @ hb_w
    by = _band_zero(W, [1.0, 2.0, 1.0]) @ hb_w
    # fold gray scale W_R into the vertical bands (gray' = R + aG + bB)
    ax *= W_R
    ay *= W_R

    # phase A: slot i at col 256*i, interleaved (s, t): col 256i+2u+t
    band_a = np.zeros((128, 1280), dtype=np.float64)
    for i, ((r0, r1), (s0, s1)) in enumerate(zip(GB, SW)):
        k, w = r1 - r0, s1 - s0
        blk = np.stack([ax[s0:s1, r0:r1], ay[s0:s1, r0:r1]], axis=-1)  # [w,k,2]
        band_a[0:k, 256 * i : 256 * i + 2 * w] = blk.transpose(1, 0, 2).reshape(
            k, 2 * w
        )

    # phase B: per (t, j) block [mj, 120] at cols (t*8+j)*120
    band_b = np.zeros((128, 2 * N_WC * 120), dtype=np.float64)
    for t, m in enumerate((bx, by)):
        for j in range(N_WC):
            w0, w1 = _wj(j)
            blk = m[120 * j : 120 * j + 120, w0:w1].T  # [mj, 120]
            band_b[0 : w1 - w0, (t * N_WC + j) * 120 : (t * N_WC + j + 1) * 120] = blk

    p4 = np.zeros((128, 30), dtype=np.float64)
    for wp in range(120):
        p4[wp, wp // 4] = 1.0 / 16.0
    return (
        band_a.astype(np.float32),
        band_b.astype(np.float32),
        p4.astype(bf16),
    )


# ------------------------------------------------------------------ bass build
def split_multi_waits(nc):
    """walrus in this container only accepts 1 sync-wait per instruction;
    hoist extra waits onto preceding same-engine NoOps."""
    for fn in nc.m.functions:
        for bb in fn.blocks:
            new_list, changed = [], False
            for ins in bb.instructions:
                si = ins.sync_info
                waits = list(si.on_wait) if si is not None else []
                if len(waits) > 1:
                    changed = True
                    for i, wt in enumerate(waits[:-1]):
                        new_list.append(
                            mybir.InstNoOp(
                                name=f"{ins.name}_ws{i}",
                                engine=ins.engine,
                                bass_nofuse=True,
                                sync_info=mybir.SyncInfo(on_wait=[wt], on_update=[]),
                            )
                        )
                    si.on_wait = [waits[-1]]
                    ins.sync_info = si
                new_list.append(ins)
            if changed:
                bb.instructions = new_list


def build_module():
    nc = bass.Bass("TRN2", target_bir_lowering=False, debug=False)
    x = nc.dram_tensor("x", [B_LOC, C, H, W], F32, kind="ExternalInput").ap()
    ba = nc.dram_tensor("bA", [128, 1280], F32, kind="ExternalInput").ap()
    bb_ = nc.dram_tensor("bB", [128, 2 * N_WC * 120], F32, kind="ExternalInput").ap()
    p4 = nc.dram_tensor("p4", [128, 30], BF16, kind="ExternalInput").ap()
    y = nc.dram_tensor("y", [B_LOC, 1, HP, WP], F32, kind="ExternalOutput").ap()

    with tile.TileContext(nc) as tc:
        with (
            tc.tile_pool(name="const", bufs=1) as cpool,
            tc.tile_pool(name="rgb", bufs=6) as rgbp,
            tc.tile_pool(name="t1", bufs=2) as t1p,
            tc.tile_pool(name="gray", bufs=2) as grayp,
            tc.tile_pool(name="xy", bufs=4) as xyp,
            tc.tile_pool(name="sq", bufs=2) as sqp,
            tc.tile_pool(name="sp", bufs=2) as spp,
            tc.tile_pool(name="outp", bufs=2) as outp,
            tc.tile_pool(name="psA1", bufs=2, space="PSUM") as psA1,
            tc.tile_pool(name="psA2", bufs=1, space="PSUM") as psA2,
            tc.tile_pool(name="psB", bufs=1, space="PSUM") as psB,
            tc.tile_pool(name="psP", bufs=1, space="PSUM") as psP,
            nc.allow_low_precision(reason="bf16 pipeline, tolerance 2e-2"),
        ):
            # ---- constants: DMA in, round to f32r (verifier requires
            # f32r-matmul inputs to be produced as rounded f32r)
            ba_raw = cpool.tile([128, 1280], F32, tag="ba_raw")
            nc.sync.dma_start(ba_raw[:], ba[:])
            ba_t = cpool.tile([128, 1280], F32, tag="ba")
            nc.vector.tensor_copy(ba_t[:].bitcast(F32R), ba_raw[:])
            bb_raw = cpool.tile([128, 2 * N_WC * 120], F32, tag="bb_raw")
            nc.sync.dma_start(bb_raw[:], bb_[:])
            bb_t = cpool.tile([128, 2 * N_WC * 120], F32, tag="bb")
            nc.vector.tensor_copy(bb_t[:].bitcast(F32R), bb_raw[:])
            p4_t = cpool.tile([128, 30], BF16, tag="p4")
            nc.sync.dma_start(p4_t[:], p4[:])

            bias_eps = cpool.tile([128, 1], F32, tag="beps")
            nc.gpsimd.memset(bias_eps[:], 1e-6)
            bias_m1 = cpool.tile([128, 1], F32, tag="bm1")
            nc.gpsimd.memset(bias_m1[:], -1.0)

            stores = []
            for b in range(B_LOC):
                # ---- gray blocks: gray' = R + (wG/wR) G + (wB/wR) B
                gray_t = []
                for i, (r0, r1) in enumerate(GB):
                    k = r1 - r0
                    rgb = rgbp.tile([128, 3 * W], F32, tag="rgb")
                    for c in range(3):
                        nc.sync.dma_start(
                            rgb[0:k, c * W : (c + 1) * W], x[b, c, r0:r1, :]
                        )
                    tr = rgb[0:k, 0:W]
                    tg = rgb[0:k, W : 2 * W]
                    tb = rgb[0:k, 2 * W : 3 * W]
                    t1 = t1p.tile([128, W], F32, tag="t1")
                    nc.vector.scalar_tensor_tensor(
                        t1[0:k, :], tg, W_G / W_R, tr, op0=ALU.mult, op1=ALU.add
                    )
                    gt = grayp.tile([128, W], F32, tag=f"g{i}")
                    nc.vector.scalar_tensor_tensor(
                        gt[0:k, :].bitcast(F32R), tb, W_B / W_R, t1[0:k, :],
                        op0=ALU.mult, op1=ALU.add,
                    )
                    gray_t.append(gt)

                pooled = psP.tile([128, 2 * WP], F32, tag="pooled")

                def stage_a(j):
                    """phase A: 5 banded matmuls into 256-aligned psum slots"""
                    w0, w1 = _wj(j)
                    mj = w1 - w0
                    a01 = psA1.tile([128, 512], F32, tag="a01")
                    a234 = psA2.tile([128, 768], F32, tag="a234")
                    for i, (r0, r1) in enumerate(GB):
                        k = r1 - r0
                        dst = a01 if i < 2 else a234
                        off = 256 * i if i < 2 else 256 * (i - 2)
                        nc.tensor.matmul(
                            dst[0:mj, off : off + 256],
                            gray_t[i][0:k, w0:w1].bitcast(F32R),
                            ba_t[0:k, 256 * i : 256 * (i + 1)].bitcast(F32R),
                            start=True,
                            stop=True,
                        )
                    return a01, a234

                def stage_copy(j, a01, a234):
                    """psum -> sbuf xy, 2 strided copies (uniform 228 slots)"""
                    w0, w1 = _wj(j)
                    mj = w1 - w0
                    xy = xyp.tile([128, 1152], F32, tag="xy")
                    nc.vector.tensor_copy(
                        xy[0:mj, 0:456].rearrange("p (b c) -> p b c", b=2)
                            .bitcast(F32R),
                        a01[0:mj, :].rearrange("p (b c) -> p b c", b=2)[
                            :, :, 0:228
                        ],
                    )
                    nc.scalar.copy(
                        xy[0:mj, 456:1140].rearrange("p (b c) -> p b c", b=3)
                            .bitcast(F32R),
                        a234[0:mj, :].rearrange("p (b c) -> p b c", b=3)[
                            :, :, 0:228
                        ],
                    )
                    return xy

                def stage_b(j, xy):
                    """phase B: gx at [224,768), gy at [768,1312) of one tile"""
                    w0, w1 = _wj(j)
                    mj = w1 - w0
                    xyv = xy[0:mj, 0:2 * H].rearrange("p (s two) -> p two s", two=2)
                    gxy = psB.tile([128, 1312], F32, tag="gxy")
                    for t, base in ((0, 224), (1, 768)):
                        bT = bb_t[
                            0:mj, (t * N_WC + j) * 120 : (t * N_WC + j + 1) * 120
                        ].bitcast(F32R)
                        n0 = 288 if t == 0 else 256
                        nc.tensor.matmul(
                            gxy[0:120, base : base + n0],
                            bT,
                            xyv[:, t, 0:n0].bitcast(F32R),
                            start=True,
                            stop=True,
                        )
                        nc.tensor.matmul(
                            gxy[0:120, base + n0 : base + 544],
                            bT,
                            xyv[:, t, n0:H].bitcast(F32R),
                            start=True,
                            stop=True,
                        )
                    return gxy

                def stage_mag1(j, gxy):
                    # one merged Square over contiguous gx|gy [120, 1088]
                    sq = sqp.tile([128, 1088], F32, tag="sq")
                    nc.scalar.activation(
                        sq[0:120, :], gxy[0:120, 224:1312], AF.Square
                    )
                    m2 = sqp.tile([128, H], F32, tag="m2")
                    nc.gpsimd.tensor_add(
                        m2[0:120, :], sq[0:120, 0:H], sq[0:120, H : 2 * H]
                    )
                    return m2

                def stage_mag2(j, m2):
                    mg = sqp.tile([128, H], BF16, tag="mg")
                    nc.scalar.activation(
                        mg[0:120, :], m2[0:120, :], AF.Sqrt, bias=bias_eps[0:120, :]
                    )
                    sp = spp.tile([128, HP], BF16, tag="sp")
                    nc.vector.tensor_reduce(
                        sp[0:120, :],
                        mg[0:120, :].rearrange("p (g f) -> p g f", f=4),
                        axis=mybir.AxisListType.X,
                        op=ALU.add,
                    )
                    return sp

                def stage_pool(j, sp):
                    nc.tensor.matmul(
                        pooled[0:96, 30 * j : 30 * j + 30],
                        sp[0:120, 0:96],
                        p4_t[0:120, :],
                        start=True,
                        stop=True,
                    )
                    nc.tensor.matmul(
                        pooled[0:40, WP + 30 * j : WP + 30 * j + 30],
                        sp[0:120, 96:HP],
                        p4_t[0:120, :],
                        start=True,
                        stop=True,
                    )

                # software-pipelined emission: PE queue order A(j+1), B(j),
                # pool(j-1) so every matmul's deps are met before queue head
                # deep software pipeline: sq/add at lag1, sqrt/reduce at
                # lag2, pool at lag3 (between A and B so PE never idles)
                aout = {0: stage_a(0)}
                m2s, sps, gxys = {}, {}, {}

                def iter_stages(j):
                    if 0 <= j < N_WC:
                        xy = stage_copy(j, *aout.pop(j))
                    if j - 1 in gxys:
                        m2s[j - 1] = stage_mag1(j - 1, gxys.pop(j - 1))
                    if j + 1 < N_WC:
                        aout[j + 1] = stage_a(j + 1)
                    if j - 3 in sps:
                        stage_pool(j - 3, sps.pop(j - 3))
                    if 0 <= j < N_WC:
                        gxys[j] = stage_b(j, xy)
                    if j - 2 in m2s:
                        sps[j - 2] = stage_mag2(j - 2, m2s.pop(j - 2))

                for j in range(N_WC + 3):
                    iter_stages(j)

                # ---- sigmoid(5x-1)^2 on pooled, store deferred to the end
                sg = outp.tile([128, 2 * WP], F32, tag="sg")
                nc.scalar.activation(
                    sg[0:96, :], pooled[0:96, :], AF.Sigmoid,
                    bias=bias_m1[0:96, :], scale=5.0,
                )
                ot = outp.tile([128, 2 * WP], F32, tag="ot")
                nc.gpsimd.tensor_mul(ot[0:96, :], sg[0:96, :], sg[0:96, :])
                stores.append((b, ot))

            # output stores last so image-1 input DMAs aren't blocked behind
            # image-0's store wait in the SP instruction stream
            for b, ot in stores:
                nc.sync.dma_start(y[b, 0, 0:96, :], ot[0:96, 0:WP])
                nc.sync.dma_start(y[b, 0, 96:HP, :], ot[0:40, WP : 2 * WP])

    split_multi_waits(nc)
    return nc


_NC = None
_CONSTS = None
TRACE = False
LAST_EXEC_NS = None


def kernel(**inputs):
    global _NC, _CONSTS, LAST_EXEC_NS
    left_rgb = np.ascontiguousarray(np.asarray(inputs["left_rgb"], dtype=np.float32))
    assert left_rgb.shape == (B_FULL, C, H, W)
    if _NC is None:
        _NC = build_module()
        _CONSTS = build_constants()
    band_a, band_b, p4 = _CONSTS
    in_maps = [
        {
            "x": np.ascontiguousarray(left_rgb[i * B_LOC : (i + 1) * B_LOC]),
            "bA": band_a,
            "bB": band_b,
            "p4": p4,
        }
        for i in range(N_CORES)
    ]
    res = run_bass_kernel_spmd(
        _NC, in_maps, core_ids=list(range(N_CORES)), trace=TRACE
    )
    LAST_EXEC_NS = res.exec_time_ns
    out = np.empty((B_FULL, 1, HP, WP), dtype=np.float32)
    for i in range(N_CORES):
        out[i * B_LOC : (i + 1) * B_LOC] = res.results[i]["y"]
    return out
